# revision 27
# baseline (speedup 1.0000x reference)
"""ArcFace logits on 8 Trainium2 NeuronCores — class-parallel (partial-FC) sharding.

Math: logits = SCALE * cos(arccos(clip(f_n @ w_n.T)) + MARGIN*onehot(targets))
Since cos(arccos(x)) == x, only the 1024 target entries need the margin
correction cos(t+m) = cos(m)*x - sin(m)*sqrt(1-x^2); everything else is just
the normalized matmul scaled by SCALE.

Device (SPMD, identical graph on all 8 cores, class-sharded; PE does ONLY the
main matmul stream):
  - w norms from a second bf16 stream of w in [class, d] layout via ACT
    Square + accum_out (free-axis reduction; classes land on partitions,
    exactly the layout the evacuation scaling needs) + DVE reciprocal + ACT
    Sqrt — no PE ones-matmuls
  - f row-normalize (*SCALE folded in), cast bf16, PE-transpose -> fT
  - main matmul out[c,b] = wT.T @ fT in bf16 (fp32 PSUM); w-norm scaling fused
    into the PSUM->SBUF evacuation (per-partition scale), output cast to bf16
    (rel-err budget is 2e-2; bf16 adds ~2e-3)
  - margin deltas for all 1024 rows from gathered target weight rows, emitted
    mid-loop so they don't sit on the kernel tail
Host: shard/transpose/concat + apply the device-computed deltas at the 1024
target positions (pure indexing; all arithmetic happens on device).
"""

import math
import os

import numpy as np

IN_F = 512
OUT_C = 100000
B = 1024
MARGIN = 0.5
SCALE = 20.0

NCORES = 8
CSH = 12800            # classes per core after padding
CPAD = CSH * NCORES    # 102400
P = 128
KT = IN_F // P         # 4 contraction subtiles
BT = B // P            # 8 batch tiles
NF = 512               # matmul moving free dim (one PSUM bank of fp32)
NB = B // NF           # 2
CW = 1280              # class chunk width streamed from DRAM
CBK = CW // P          # 10 class blocks per chunk
CHUNKS = CSH // CW     # 10
OG = 5                 # c-blocks batched per output DMA

_GRAPH = None
LAST_EXEC_TIME_NS = None
LAST_RES = None


def _build_graph():
    from contextlib import ExitStack

    import concourse.bass as bass  # noqa: F401
    import concourse.tile as tile
    from concourse import bacc, mybir
    from concourse.masks import make_identity

    dt = mybir.dt
    AF = mybir.ActivationFunctionType
    ALU = mybir.AluOpType
    cosm = math.cos(MARGIN)
    sinm = math.sin(MARGIN)

    nc = bacc.Bacc()
    wT_e = nc.declare_dram_parameter("wT", [IN_F, CSH], dt.bfloat16, isOutput=False)
    wn_e = nc.declare_dram_parameter("wn", [CSH, IN_F], dt.bfloat16, isOutput=False)
    f_e = nc.declare_dram_parameter("f", [B, IN_F], dt.float32, isOutput=False)
    wtg_e = nc.declare_dram_parameter("wtgt", [B, IN_F], dt.float32, isOutput=False)
    out_e = nc.declare_dram_parameter("out", [CSH, B], dt.bfloat16, isOutput=True)
    dlt_e = nc.declare_dram_parameter("delta", [P, BT], dt.float32, isOutput=True)

    wT_v = wT_e[:].rearrange("(k p) c -> p k c", p=P)   # d = k*128 + p
    wn_v = wn_e[:].rearrange("(n p) d -> p n d", p=P)   # c = n*128 + p
    f_v = f_e[:].rearrange("(t p) d -> p t d", p=P)     # b = t*128 + p
    wtg_v = wtg_e[:].rearrange("(t p) d -> p t d", p=P)

    with ExitStack() as ctx:
        tc = ctx.enter_context(tile.TileContext(nc))
        cpool = ctx.enter_context(tc.tile_pool(name="cpool", bufs=1))
        fpool = ctx.enter_context(tc.tile_pool(name="fpool", bufs=1))
        wpool = ctx.enter_context(tc.tile_pool(name="wpool", bufs=3))
        wnpool = ctx.enter_context(tc.tile_pool(name="wnpool", bufs=3))
        sqpool = ctx.enter_context(tc.tile_pool(name="sqpool", bufs=3))
        opool = ctx.enter_context(tc.tile_pool(name="opool", bufs=4))
        smal = ctx.enter_context(tc.tile_pool(name="smal", bufs=2))
        pt_pool = ctx.enter_context(tc.tile_pool(name="pt", bufs=2, space="PSUM"))
        po_pool = ctx.enter_context(tc.tile_pool(name="po", bufs=6, space="PSUM"))

        ident = cpool.tile([P, P], dt.bfloat16)
        make_identity(nc, ident[:])

        # pre-warm the ACT table sets (Sqrt first — it gates rnf20 on the
        # critical path — then Square) during the initial DMA window;
        # otherwise the ~1.5us ACT_TABLE_LOADs land at first real use
        twarm = cpool.tile([P, 1], dt.float32)
        nc.gpsimd.memset(twarm[:], 1.0)
        twout = cpool.tile([P, 1], dt.float32)
        nc.scalar.activation(twout[:], twarm[:], AF.Sqrt, scale=1.0)
        nc.scalar.activation(twout[:], twarm[:], AF.Square)

        # ---------------- f path: normalize, *SCALE, cast bf16, transpose ---
        # two sequential DMAs on the sync ring: the first half lands first so
        # the norm squares start while the second half streams.  NOTHING else
        # is in flight while f streams: the weight loads below are issued
        # from the ACT queue BETWEEN the f squares, so each issue fires only
        # after the f data it would otherwise compete with has landed (the
        # HWDGE ring runs ~4 transfers concurrently — ring order is NOT a
        # bandwidth priority; ACT-queue serialization is the timing gate).
        f_sb = fpool.tile([P, BT, IN_F], dt.float32)
        nc.sync.dma_start(f_sb[:, : BT // 2], f_v[:, : BT // 2])
        nc.sync.dma_start(f_sb[:, BT // 2 :], f_v[:, BT // 2 :])

        # pre-allocate the first three chunks' tiles; their DMAs are gated
        # into the f path below
        w_tiles = {
            ci: (
                wpool.tile([P, KT, CW], dt.bfloat16, tag="wchunk", name="w_sb"),
                wnpool.tile([P, CBK, IN_F], dt.bfloat16, tag="wnchunk", name="wn_sb"),
            )
            for ci in range(3)
        }

        def issue_wT(ci, eng):
            eng.dma_start(w_tiles[ci][0][:], wT_v[:, :, ci * CW : (ci + 1) * CW])

        def issue_wn(ci, eng):
            eng.dma_start(w_tiles[ci][1][:], wn_v[:, ci * CBK : (ci + 1) * CBK])

        # f squares and scale-muls split across ACT and DVE to halve the
        # serial startup chain
        nf2 = smal.tile([P, BT], dt.float32)
        for t in range(BT):
            if t % 2 == 0:
                sq = sqpool.tile([P, IN_F], dt.bfloat16, tag="sqscratch")
                nc.scalar.activation(
                    sq[:], f_sb[:, t], AF.Square, accum_out=nf2[:, t : t + 1]
                )
            else:
                prod = sqpool.tile([P, IN_F], dt.float32, tag="prodscratch")
                nc.vector.tensor_mul(prod[:], f_sb[:, t], f_sb[:, t])
                nc.vector.reduce_sum(
                    nf2[:, t : t + 1], prod[:], axis=mybir.AxisListType.X
                )
            if t == 0:
                issue_wT(0, nc.scalar)
            elif t == 2:
                issue_wn(0, nc.scalar)
            elif t == 4:
                issue_wT(1, nc.scalar)
            elif t == 6:
                issue_wn(1, nc.scalar)
        rec_f = smal.tile([P, BT], dt.float32)
        nc.vector.reciprocal(rec_f[:], nf2[:])
        rnf20 = smal.tile([P, BT], dt.float32)
        # sqrt(SCALE^2 / nf2) = SCALE * rsqrt(nf2)
        nc.scalar.activation(rnf20[:], rec_f[:], AF.Sqrt, scale=SCALE * SCALE)

        f_n = fpool.tile([P, BT, IN_F], dt.bfloat16)
        for t in range(BT):
            if t % 2 == 0:
                nc.vector.tensor_scalar_mul(f_n[:, t], f_sb[:, t], rnf20[:, t : t + 1])
            else:
                nc.scalar.activation(
                    f_n[:, t], f_sb[:, t], AF.Copy, scale=rnf20[:, t : t + 1]
                )
            if t == 1:
                issue_wT(2, nc.scalar)
            elif t == 3:
                issue_wn(2, nc.scalar)

        fT = fpool.tile([P, KT, B], dt.bfloat16)
        for t in range(BT):
            for k in range(KT):
                ps = pt_pool.tile([P, P], dt.bfloat16, tag="pst")
                nc.tensor.transpose(ps[:], f_n[:, t, k * P : (k + 1) * P], ident[:])
                nc.vector.tensor_copy(fT[:, k, t * P : (t + 1) * P], ps[:])

        # ---------------- w chunk streams -----------------------------------
        def emit_load(ci):
            """DMA one chunk of wT (matmul layout) and wn (norm layout)."""
            if ci < 3:
                # DMAs already issued from the gated startup sequence
                return w_tiles[ci]
            w_sb = wpool.tile([P, KT, CW], dt.bfloat16, tag="wchunk", name="w_sb")
            wn_sb = wnpool.tile([P, CBK, IN_F], dt.bfloat16, tag="wnchunk", name="wn_sb")
            # ch3+ on SWDGE: their pool-slot WAR deps (bufs=3) self-time them
            # one chunk-period ahead, so they never compete with startup
            nc.gpsimd.dma_start(w_sb[:], wT_v[:, :, ci * CW : (ci + 1) * CW])
            nc.gpsimd.dma_start(wn_sb[:], wn_v[:, ci * CBK : (ci + 1) * CBK])
            return w_sb, wn_sb

        def emit_norm(wn_sb):
            """Per-class 1/||w|| for one chunk -> [128, CBK], classes on partitions.

            First half on ACT (Square+accum), second half on DVE (mul+reduce)
            to balance engine load; reciprocal/sqrt per half so the first
            evacuations of the chunk don't wait on the whole chunk's norms."""
            rnw = smal.tile([P, CBK], dt.float32, tag="rnw", name="rnw")
            half = CBK // 2
            nw2 = smal.tile([P, CBK], dt.float32, tag="nw2", name="nw2")
            for g in range(CBK):
                if g < half:
                    sq = sqpool.tile([P, IN_F], dt.bfloat16, tag="sqscratch")
                    nc.scalar.activation(
                        sq[:], wn_sb[:, g], AF.Square, accum_out=nw2[:, g : g + 1]
                    )
                else:
                    prod = sqpool.tile([P, IN_F], dt.float32, tag="prodscratch")
                    nc.vector.tensor_mul(prod[:], wn_sb[:, g], wn_sb[:, g])
                    nc.vector.reduce_sum(
                        nw2[:, g : g + 1], prod[:], axis=mybir.AxisListType.X
                    )
                if g == half - 1:
                    recw = smal.tile([P, half], dt.float32, tag="recw", name="recw")
                    nc.vector.reciprocal(recw[:], nw2[:, :half])
                    nc.scalar.activation(rnw[:, :half], recw[:], AF.Sqrt, scale=1.0)
            recw2 = smal.tile([P, CBK - half], dt.float32, tag="recw2", name="recw2")
            nc.vector.reciprocal(recw2[:], nw2[:, half:])
            nc.scalar.activation(rnw[:, half:], recw2[:], AF.Sqrt, scale=1.0)
            return rnw

        # margin input DMA deferred to mid-loop
        wt_sb = fpool.tile([P, BT, IN_F], dt.float32, name="wt_sb")
        nt2 = smal.tile([P, BT], dt.float32, name="nt2")
        drot = smal.tile([P, BT], dt.float32, name="drot")

        def emit_margin_dots(ts):
            for t in ts:
                sq = sqpool.tile([P, IN_F], dt.bfloat16, tag="sqscratch")
                nc.scalar.activation(
                    sq[:], wt_sb[:, t], AF.Square, accum_out=nt2[:, t : t + 1]
                )
                prod = sqpool.tile([P, IN_F], dt.float32, tag="prodscratch")
                nc.vector.tensor_mul(prod[:], f_sb[:, t], wt_sb[:, t])
                nc.vector.reduce_sum(
                    drot[:, t : t + 1], prod[:], axis=mybir.AxisListType.X
                )

        def emit_margin():
            rec_t = smal.tile([P, BT], dt.float32)
            nc.vector.reciprocal(rec_t[:], nt2[:])
            rnt = smal.tile([P, BT], dt.float32)
            nc.scalar.activation(rnt[:], rec_t[:], AF.Sqrt, scale=1.0)
            u = smal.tile([P, BT], dt.float32)
            nc.vector.tensor_mul(u[:], drot[:], rnf20[:])
            nc.vector.tensor_mul(u[:], u[:], rnt[:])          # u = SCALE * cos_t
            t1 = smal.tile([P, BT], dt.float32)
            nc.vector.tensor_mul(t1[:], u[:], u[:])
            nc.vector.tensor_scalar(t1[:], t1[:], -1.0, SCALE * SCALE, ALU.mult, ALU.add)
            nc.vector.tensor_scalar_max(t1[:], t1[:], 0.0)    # max(S^2 - u^2, 0)
            s_t = smal.tile([P, BT], dt.float32)
            nc.scalar.activation(s_t[:], t1[:], AF.Sqrt, scale=1.0)  # SCALE*sin_t
            t2 = smal.tile([P, BT], dt.float32)
            nc.vector.tensor_scalar_mul(t2[:], s_t[:], -sinm)
            t3 = smal.tile([P, BT], dt.float32)
            nc.vector.tensor_scalar_mul(t3[:], u[:], cosm - 1.0)
            delta = smal.tile([P, BT], dt.float32)
            nc.vector.tensor_add(delta[:], t2[:], t3[:])
            nc.sync.dma_start(dlt_e[:], delta[:])

        # ---------------- main class loop ------------------------------------
        ready = {}   # ci -> (w_sb, rnw)
        raw = {}     # ci -> (w_sb, wn_sb)
        w_sb0, wn_sb0 = emit_load(0)
        ready[0] = (w_sb0, emit_norm(wn_sb0))
        raw[1] = emit_load(1)

        for ci in range(CHUNKS):
            w_sb, rnw = ready.pop(ci)
            for og in range(CBK // OG):
                if og == 0 and ci + 2 < CHUNKS:
                    raw[ci + 2] = emit_load(ci + 2)
                if og == 1 and ci + 1 in raw:
                    # norm compute for the next chunk lands between this
                    # chunk's output groups, well before it's consumed
                    nw_sb, nwn_sb = raw.pop(ci + 1)
                    ready[ci + 1] = (nw_sb, emit_norm(nwn_sb))
                if og == 0 and ci == 4:
                    # margin input: SWDGE, issued after ch6's prefetch so the
                    # scheduler doesn't hoist the margin math over the f path
                    nc.gpsimd.dma_start(wt_sb[:], wtg_v)
                if og == 0 and ci == 5:
                    emit_margin_dots(range(0, BT // 2))
                if og == 0 and ci == 6:
                    emit_margin_dots(range(BT // 2, BT))
                    emit_margin()
                osb = opool.tile([P, OG, B], dt.bfloat16, tag="osb")
                for cbi in range(OG):
                    cb = og * OG + cbi
                    psos = [
                        po_pool.tile([P, NF], dt.float32, tag="pso", name=f"pso{nb}")
                        for nb in range(NB)
                    ]
                    for k in range(KT):
                        for nb in range(NB):
                            nc.tensor.matmul(
                                psos[nb][:],
                                lhsT=w_sb[:, k, cb * P : (cb + 1) * P],
                                rhs=fT[:, k, nb * NF : (nb + 1) * NF],
                                start=(k == 0),
                                stop=(k == KT - 1),
                            )
                    for nb in range(NB):
                        eidx = (ci * CBK + cb) * NB + nb
                        if eidx % 5 < 3:
                            nc.scalar.activation(
                                osb[:, cbi, nb * NF : (nb + 1) * NF],
                                psos[nb][:], AF.Copy,
                                scale=rnw[:, cb : cb + 1],
                            )
                        else:
                            nc.vector.tensor_scalar_mul(
                                osb[:, cbi, nb * NF : (nb + 1) * NF],
                                psos[nb][:], rnw[:, cb : cb + 1],
                            )
                row0 = ci * CW + og * OG * P
                # outputs on the sync ring (no compute there, so issues/waits
                # never stall an engine queue) — EXCEPT the final group, which
                # goes on the (empty-at-tail) scalar ring so it doesn't queue
                # behind the previous group's still-draining transfer
                last_group = ci == CHUNKS - 1 and og == (CBK // OG) - 1
                dma_eng = nc.scalar if last_group else nc.sync
                dma_eng.dma_start(
                    out_e[row0 : row0 + OG * P, :].rearrange(
                        "(g p) b -> p g b", p=P
                    ),
                    osb[:],
                )

    nc.finalize()
    return nc


def _prep_inputs(features, targets, weights):
    import ml_dtypes

    f32 = np.ascontiguousarray(np.asarray(features, dtype=np.float32))
    tgt = np.asarray(targets).astype(np.int64)
    w = np.asarray(weights, dtype=np.float32)

    wpad = np.zeros((CPAD, IN_F), dtype=np.float32)
    wpad[:OUT_C] = w
    wpad[OUT_C:, 0] = 1.0  # unit-norm filler rows: no inf/nan anywhere

    in_maps = []
    for i in range(NCORES):
        sh = wpad[i * CSH : (i + 1) * CSH]
        shb = sh.astype(ml_dtypes.bfloat16)
        wT = np.ascontiguousarray(shb.T)
        loc = np.clip(tgt - i * CSH, 0, CSH - 1)
        wtgt = np.ascontiguousarray(sh[loc])
        in_maps.append({"wT": wT, "wn": shb, "f": f32, "wtgt": wtgt})
    return in_maps, tgt


def kernel(features, targets, weights):
    global _GRAPH, LAST_EXEC_TIME_NS, LAST_RES
    from concourse.bass_utils import run_bass_kernel_spmd

    if _GRAPH is None:
        _GRAPH = _build_graph()
    nc = _GRAPH

    in_maps, tgt = _prep_inputs(features, targets, weights)

    trace = bool(int(os.environ.get("BASS_KERNEL_TRACE", "0")))
    res = run_bass_kernel_spmd(nc, in_maps, core_ids=list(range(NCORES)), trace=trace)
    LAST_EXEC_TIME_NS = res.exec_time_ns
    LAST_RES = res

    outs = [res.results[i]["out"] for i in range(NCORES)]       # [CSH, B] bf16 each
    full = np.concatenate(outs, axis=0)[:OUT_C]                 # [OUT_C, B] bf16
    logits = np.ascontiguousarray(full.T, dtype=np.float32)     # [B, OUT_C] f32

    # apply device-computed margin deltas at the 1024 target positions
    deltas = np.stack(
        [res.results[i]["delta"].T.reshape(B) for i in range(NCORES)]
    )  # [NCORES, B]; delta[p, t] -> b = t*128 + p
    rows = np.arange(B)
    core_of = (tgt // CSH).astype(np.int64)
    logits[rows, tgt] += deltas[core_of, rows]
    return logits


# revision 32
# speedup vs baseline: 1.0875x; 1.0875x over previous
"""ArcFace logits on 8 Trainium2 NeuronCores — class-parallel (partial-FC) sharding.

Math: logits = SCALE * cos(arccos(clip(f_n @ w_n.T)) + MARGIN*onehot(targets))
Since cos(arccos(x)) == x, only the 1024 target entries need the margin
correction cos(t+m) = cos(m)*x - sin(m)*sqrt(1-x^2); everything else is just
the normalized matmul scaled by SCALE.

Device (SPMD, identical graph on all 8 cores, class-sharded; PE does ONLY the
main matmul stream):
  - w norms from a second bf16 stream of w in [class, d] layout via ACT
    Square + accum_out (free-axis reduction; classes land on partitions,
    exactly the layout the evacuation scaling needs) + DVE reciprocal + ACT
    Sqrt — no PE ones-matmuls
  - f row-normalize (*SCALE folded in), cast bf16, PE-transpose -> fT
  - main matmul out[c,b] = wT.T @ fT in bf16 (fp32 PSUM); w-norm scaling fused
    into the PSUM->SBUF evacuation (per-partition scale), output cast to bf16
    (rel-err budget is 2e-2; bf16 adds ~2e-3)
  - margin deltas for all 1024 rows from gathered target weight rows, emitted
    mid-loop so they don't sit on the kernel tail
Host: shard/transpose/concat + apply the device-computed deltas at the 1024
target positions (pure indexing; all arithmetic happens on device).
"""

import math
import os

import numpy as np

IN_F = 512
OUT_C = 100000
B = 1024
MARGIN = 0.5
SCALE = 20.0

NCORES = 8
CSH = 12800            # classes per core after padding
CPAD = CSH * NCORES    # 102400
P = 128
KT = IN_F // P         # 4 contraction subtiles
BT = B // P            # 8 batch tiles
NF = 512               # matmul moving free dim (one PSUM bank of fp32)
NB = B // NF           # 2
CW = 1280              # class chunk width streamed from DRAM
CBK = CW // P          # 10 class blocks per chunk
CHUNKS = CSH // CW     # 10
OG = 5                 # c-blocks batched per output DMA

_GRAPH = None
LAST_EXEC_TIME_NS = None
LAST_RES = None


def _build_graph():
    from contextlib import ExitStack

    import concourse.bass as bass  # noqa: F401
    import concourse.tile as tile
    from concourse import bacc, mybir
    from concourse.masks import make_identity

    dt = mybir.dt
    AF = mybir.ActivationFunctionType
    ALU = mybir.AluOpType
    cosm = math.cos(MARGIN)
    sinm = math.sin(MARGIN)

    nc = bacc.Bacc()
    wT_e = nc.declare_dram_parameter("wT", [IN_F, CSH], dt.bfloat16, isOutput=False)
    wn_e = nc.declare_dram_parameter("wn", [CSH, IN_F], dt.bfloat16, isOutput=False)
    f_e = nc.declare_dram_parameter("f", [B, IN_F], dt.bfloat16, isOutput=False)
    wtg_e = nc.declare_dram_parameter("wtgt", [B, IN_F], dt.bfloat16, isOutput=False)
    out_e = nc.declare_dram_parameter("out", [CSH, B], dt.bfloat16, isOutput=True)
    dlt_e = nc.declare_dram_parameter("delta", [P, BT], dt.float32, isOutput=True)

    wT_v = wT_e[:].rearrange("(k p) c -> p k c", p=P)   # d = k*128 + p
    wn_v = wn_e[:].rearrange("(n p) d -> p n d", p=P)   # c = n*128 + p
    f_v = f_e[:].rearrange("(t p) d -> p t d", p=P)     # b = t*128 + p
    wtg_v = wtg_e[:].rearrange("(t p) d -> p t d", p=P)

    with ExitStack() as ctx:
        tc = ctx.enter_context(tile.TileContext(nc))
        cpool = ctx.enter_context(tc.tile_pool(name="cpool", bufs=1))
        fpool = ctx.enter_context(tc.tile_pool(name="fpool", bufs=1))
        wpool = ctx.enter_context(tc.tile_pool(name="wpool", bufs=3))
        wnpool = ctx.enter_context(tc.tile_pool(name="wnpool", bufs=3))
        sqpool = ctx.enter_context(tc.tile_pool(name="sqpool", bufs=3))
        opool = ctx.enter_context(tc.tile_pool(name="opool", bufs=4))
        smal = ctx.enter_context(tc.tile_pool(name="smal", bufs=2))
        pt_pool = ctx.enter_context(tc.tile_pool(name="pt", bufs=2, space="PSUM"))
        po_pool = ctx.enter_context(tc.tile_pool(name="po", bufs=6, space="PSUM"))

        ident = cpool.tile([P, P], dt.bfloat16)
        make_identity(nc, ident[:])

        # pre-warm the ACT table sets (Sqrt first — it gates rnf20 on the
        # critical path — then Square) during the initial DMA window;
        # otherwise the ~1.5us ACT_TABLE_LOADs land at first real use
        twarm = cpool.tile([P, 1], dt.float32)
        nc.gpsimd.memset(twarm[:], 1.0)
        twout = cpool.tile([P, 1], dt.float32)
        nc.scalar.activation(twout[:], twarm[:], AF.Sqrt, scale=1.0)
        nc.scalar.activation(twout[:], twarm[:], AF.Square)

        # ---------------- f path: normalize, *SCALE, cast bf16, transpose ---
        # f ships as bf16 (1MB) and rides the sync HWDGE ring ALONE — the
        # weight streams all ride SWDGE, whose per-SDMA-engine queues
        # round-robin against the HWDGE queue, so f keeps ~half the
        # bandwidth no matter how much weight traffic is in flight
        f_sb = fpool.tile([P, BT, IN_F], dt.bfloat16)
        nc.sync.dma_start(f_sb[:, : BT // 2], f_v[:, : BT // 2])
        nc.sync.dma_start(f_sb[:, BT // 2 :], f_v[:, BT // 2 :])

        # f squares and scale-muls split across ACT and DVE to halve the
        # serial startup chain
        nf2 = smal.tile([P, BT], dt.float32)
        for t in range(BT):
            if t % 2 == 0:
                sq = sqpool.tile([P, IN_F], dt.bfloat16, tag="sqscratch")
                nc.scalar.activation(
                    sq[:], f_sb[:, t], AF.Square, accum_out=nf2[:, t : t + 1]
                )
            else:
                prod = sqpool.tile([P, IN_F], dt.float32, tag="prodscratch")
                nc.vector.tensor_mul(prod[:], f_sb[:, t], f_sb[:, t])
                nc.vector.reduce_sum(
                    nf2[:, t : t + 1], prod[:], axis=mybir.AxisListType.X
                )
        rec_f = smal.tile([P, BT], dt.float32)
        nc.vector.reciprocal(rec_f[:], nf2[:])
        rnf20 = smal.tile([P, BT], dt.float32)
        # sqrt(SCALE^2 / nf2) = SCALE * rsqrt(nf2)
        nc.scalar.activation(rnf20[:], rec_f[:], AF.Sqrt, scale=SCALE * SCALE)

        f_n = fpool.tile([P, BT, IN_F], dt.bfloat16)
        for t in range(BT):
            if t % 2 == 0:
                nc.vector.tensor_scalar_mul(f_n[:, t], f_sb[:, t], rnf20[:, t : t + 1])
            else:
                nc.scalar.activation(
                    f_n[:, t], f_sb[:, t], AF.Copy, scale=rnf20[:, t : t + 1]
                )

        fT = fpool.tile([P, KT, B], dt.bfloat16)
        for t in range(BT):
            for k in range(KT):
                ps = pt_pool.tile([P, P], dt.bfloat16, tag="pst")
                nc.tensor.transpose(ps[:], f_n[:, t, k * P : (k + 1) * P], ident[:])
                nc.vector.tensor_copy(fT[:, k, t * P : (t + 1) * P], ps[:])

        # ---------------- w chunk streams -----------------------------------
        def emit_load(ci):
            """DMA one chunk of wT (matmul layout) and wn (norm layout)."""
            w_sb = wpool.tile([P, KT, CW], dt.bfloat16, tag="wchunk", name="w_sb")
            wn_sb = wnpool.tile([P, CBK, IN_F], dt.bfloat16, tag="wnchunk", name="wn_sb")
            # all weight streams on SWDGE (see note above on the f load)
            nc.gpsimd.dma_start(w_sb[:], wT_v[:, :, ci * CW : (ci + 1) * CW])
            nc.gpsimd.dma_start(wn_sb[:], wn_v[:, ci * CBK : (ci + 1) * CBK])
            return w_sb, wn_sb

        def emit_norm(wn_sb):
            """Per-class 1/||w|| for one chunk -> [128, CBK], classes on partitions.

            First half on ACT (Square+accum), second half on DVE (mul+reduce)
            to balance engine load; reciprocal/sqrt per half so the first
            evacuations of the chunk don't wait on the whole chunk's norms."""
            rnw = smal.tile([P, CBK], dt.float32, tag="rnw", name="rnw")
            half = CBK // 2
            nw2 = smal.tile([P, CBK], dt.float32, tag="nw2", name="nw2")
            for g in range(CBK):
                if g < half:
                    sq = sqpool.tile([P, IN_F], dt.bfloat16, tag="sqscratch")
                    nc.scalar.activation(
                        sq[:], wn_sb[:, g], AF.Square, accum_out=nw2[:, g : g + 1]
                    )
                else:
                    prod = sqpool.tile([P, IN_F], dt.float32, tag="prodscratch")
                    nc.vector.tensor_mul(prod[:], wn_sb[:, g], wn_sb[:, g])
                    nc.vector.reduce_sum(
                        nw2[:, g : g + 1], prod[:], axis=mybir.AxisListType.X
                    )
                if g == half - 1:
                    recw = smal.tile([P, half], dt.float32, tag="recw", name="recw")
                    nc.vector.reciprocal(recw[:], nw2[:, :half])
                    nc.scalar.activation(rnw[:, :half], recw[:], AF.Sqrt, scale=1.0)
            recw2 = smal.tile([P, CBK - half], dt.float32, tag="recw2", name="recw2")
            nc.vector.reciprocal(recw2[:], nw2[:, half:])
            nc.scalar.activation(rnw[:, half:], recw2[:], AF.Sqrt, scale=1.0)
            return rnw

        # margin input DMA deferred to mid-loop
        wt_sb = fpool.tile([P, BT, IN_F], dt.bfloat16, name="wt_sb")
        nt2 = smal.tile([P, BT], dt.float32, name="nt2")
        drot = smal.tile([P, BT], dt.float32, name="drot")

        def emit_margin_dots(ts):
            for t in ts:
                sq = sqpool.tile([P, IN_F], dt.bfloat16, tag="sqscratch")
                nc.scalar.activation(
                    sq[:], wt_sb[:, t], AF.Square, accum_out=nt2[:, t : t + 1]
                )
                prod = sqpool.tile([P, IN_F], dt.float32, tag="prodscratch")
                nc.vector.tensor_mul(prod[:], f_sb[:, t], wt_sb[:, t])
                nc.vector.reduce_sum(
                    drot[:, t : t + 1], prod[:], axis=mybir.AxisListType.X
                )

        def emit_margin():
            rec_t = smal.tile([P, BT], dt.float32)
            nc.vector.reciprocal(rec_t[:], nt2[:])
            rnt = smal.tile([P, BT], dt.float32)
            nc.scalar.activation(rnt[:], rec_t[:], AF.Sqrt, scale=1.0)
            u = smal.tile([P, BT], dt.float32)
            nc.vector.tensor_mul(u[:], drot[:], rnf20[:])
            nc.vector.tensor_mul(u[:], u[:], rnt[:])          # u = SCALE * cos_t
            t1 = smal.tile([P, BT], dt.float32)
            nc.vector.tensor_mul(t1[:], u[:], u[:])
            nc.vector.tensor_scalar(t1[:], t1[:], -1.0, SCALE * SCALE, ALU.mult, ALU.add)
            nc.vector.tensor_scalar_max(t1[:], t1[:], 0.0)    # max(S^2 - u^2, 0)
            s_t = smal.tile([P, BT], dt.float32)
            nc.scalar.activation(s_t[:], t1[:], AF.Sqrt, scale=1.0)  # SCALE*sin_t
            t2 = smal.tile([P, BT], dt.float32)
            nc.vector.tensor_scalar_mul(t2[:], s_t[:], -sinm)
            t3 = smal.tile([P, BT], dt.float32)
            nc.vector.tensor_scalar_mul(t3[:], u[:], cosm - 1.0)
            delta = smal.tile([P, BT], dt.float32)
            nc.vector.tensor_add(delta[:], t2[:], t3[:])
            nc.sync.dma_start(dlt_e[:], delta[:])

        # ---------------- main class loop ------------------------------------
        ready = {}   # ci -> (w_sb, rnw)
        raw = {}     # ci -> (w_sb, wn_sb)
        w_sb0, wn_sb0 = emit_load(0)
        ready[0] = (w_sb0, emit_norm(wn_sb0))
        raw[1] = emit_load(1)

        for ci in range(CHUNKS):
            w_sb, rnw = ready.pop(ci)
            for og in range(CBK // OG):
                if og == 0 and ci + 2 < CHUNKS:
                    raw[ci + 2] = emit_load(ci + 2)
                if og == 1 and ci + 1 in raw:
                    # norm compute for the next chunk lands between this
                    # chunk's output groups, well before it's consumed
                    nw_sb, nwn_sb = raw.pop(ci + 1)
                    ready[ci + 1] = (nw_sb, emit_norm(nwn_sb))
                if og == 0 and ci == 4:
                    # margin input: SWDGE, issued after ch6's prefetch so the
                    # scheduler doesn't hoist the margin math over the f path
                    nc.gpsimd.dma_start(wt_sb[:], wtg_v)
                if og == 0 and ci == 5:
                    emit_margin_dots(range(0, BT // 2))
                if og == 0 and ci == 6:
                    emit_margin_dots(range(BT // 2, BT))
                    emit_margin()
                osb = opool.tile([P, OG, B], dt.bfloat16, tag="osb")
                for cbi in range(OG):
                    cb = og * OG + cbi
                    psos = [
                        po_pool.tile([P, NF], dt.float32, tag="pso", name=f"pso{nb}")
                        for nb in range(NB)
                    ]
                    for k in range(KT):
                        for nb in range(NB):
                            nc.tensor.matmul(
                                psos[nb][:],
                                lhsT=w_sb[:, k, cb * P : (cb + 1) * P],
                                rhs=fT[:, k, nb * NF : (nb + 1) * NF],
                                start=(k == 0),
                                stop=(k == KT - 1),
                            )
                    for nb in range(NB):
                        eidx = (ci * CBK + cb) * NB + nb
                        if eidx % 5 < 3:
                            nc.scalar.activation(
                                osb[:, cbi, nb * NF : (nb + 1) * NF],
                                psos[nb][:], AF.Copy,
                                scale=rnw[:, cb : cb + 1],
                            )
                        else:
                            nc.vector.tensor_scalar_mul(
                                osb[:, cbi, nb * NF : (nb + 1) * NF],
                                psos[nb][:], rnw[:, cb : cb + 1],
                            )
                row0 = ci * CW + og * OG * P
                # outputs on the sync ring (no compute there, so issues/waits
                # never stall an engine queue) — EXCEPT the final group, which
                # goes on the (empty-at-tail) scalar ring so it doesn't queue
                # behind the previous group's still-draining transfer
                last_group = ci == CHUNKS - 1 and og == (CBK // OG) - 1
                dma_eng = nc.scalar if last_group else nc.sync
                dma_eng.dma_start(
                    out_e[row0 : row0 + OG * P, :].rearrange(
                        "(g p) b -> p g b", p=P
                    ),
                    osb[:],
                )

    nc.finalize()
    return nc


def _prep_inputs(features, targets, weights):
    import ml_dtypes

    f32 = np.ascontiguousarray(np.asarray(features, dtype=np.float32))
    tgt = np.asarray(targets).astype(np.int64)
    w = np.asarray(weights, dtype=np.float32)

    wpad = np.zeros((CPAD, IN_F), dtype=np.float32)
    wpad[:OUT_C] = w
    wpad[OUT_C:, 0] = 1.0  # unit-norm filler rows: no inf/nan anywhere

    fb = f32.astype(ml_dtypes.bfloat16)
    in_maps = []
    for i in range(NCORES):
        sh = wpad[i * CSH : (i + 1) * CSH]
        shb = sh.astype(ml_dtypes.bfloat16)
        wT = np.ascontiguousarray(shb.T)
        loc = np.clip(tgt - i * CSH, 0, CSH - 1)
        wtgt = np.ascontiguousarray(shb[loc])
        in_maps.append({"wT": wT, "wn": shb, "f": fb, "wtgt": wtgt})
    return in_maps, tgt


def kernel(features, targets, weights):
    global _GRAPH, LAST_EXEC_TIME_NS, LAST_RES
    from concourse.bass_utils import run_bass_kernel_spmd

    if _GRAPH is None:
        _GRAPH = _build_graph()
    nc = _GRAPH

    in_maps, tgt = _prep_inputs(features, targets, weights)

    trace = bool(int(os.environ.get("BASS_KERNEL_TRACE", "0")))
    res = run_bass_kernel_spmd(nc, in_maps, core_ids=list(range(NCORES)), trace=trace)
    LAST_EXEC_TIME_NS = res.exec_time_ns
    LAST_RES = res

    outs = [res.results[i]["out"] for i in range(NCORES)]       # [CSH, B] bf16 each
    full = np.concatenate(outs, axis=0)[:OUT_C]                 # [OUT_C, B] bf16
    logits = np.ascontiguousarray(full.T, dtype=np.float32)     # [B, OUT_C] f32

    # apply device-computed margin deltas at the 1024 target positions
    deltas = np.stack(
        [res.results[i]["delta"].T.reshape(B) for i in range(NCORES)]
    )  # [NCORES, B]; delta[p, t] -> b = t*128 + p
    rows = np.arange(B)
    core_of = (tgt // CSH).astype(np.int64)
    logits[rows, tgt] += deltas[core_of, rows]
    return logits
